# revision 1
# baseline (speedup 1.0000x reference)
"""Bahdanau additive attention, data-parallel over batch on 8 TRN2 NeuronCores.

Math (per batch row b):
    dec_proj = W @ prev[b] + b_W                       # [A]   (computed on host: tiny)
    enc_proj[s] = U @ enc[b,s] + b_U                   # [S, A]
    energy[s] = v . tanh(dec_proj + enc_proj[s])       # [S]
    w = exp(energy);  c[b] = (w @ enc[b]) / sum(w)     # [CTX]

Device strategy (per core, 8 batches):
  - enc passed as bf16.  For each 128-row s-tile:
      * natural DMA      -> [s=128, c=1024]  (rhs for the weighted-sum matmul)
      * XBAR transpose   -> [c=128 x 8, s=128] (stationary lhsT for the U-projection)
  - U-projection: psum[s,A] = sum_k trT[c_k,s].T @ UT[c_k,A], bias folded in via a
    K=1 ones-matmul with rhs = (dec_proj[b] + b_U).
  - tanh on ScalarE, v-weighting on VectorE, sum_a via ScalarE Copy+accum.
  - exp -> w (bf16); weighted sum + denominator accumulate in PSUM across the
    32 s-tiles of a batch:  c_psum[1,512]x2 += w.T @ enc_tile, den += w.T @ ones.
  - epilogue per batch: c = c_psum * (1/den), DMA out fp32.
One HBM pass of enc per layout (2 reads of the bf16 tensor total).
"""

import sys

sys.path.insert(0, "/opt/trn_rl_repo")

import numpy as np
import ml_dtypes

import concourse.bass as bass
from concourse import bacc
import concourse.mybir as mybir
import concourse.tile as tile
from concourse.bass_utils import run_bass_kernel_spmd

B, S, A, DD, CTX = 64, 4096, 256, 1024, 1024
NCORES = 8
BL = B // NCORES  # 8 batches per core
P = 128
KC = CTX // P  # 8 contraction chunks
MT = S // P    # 32 s-tiles per batch
BF16 = mybir.dt.bfloat16
F32 = mybir.dt.float32

_CACHE = {}


def _fast_bf16(x: np.ndarray) -> np.ndarray:
    """float32 -> bfloat16 with round-to-nearest-even via integer ops
    (ml_dtypes.astype is ~50x slower on GiB-scale arrays)."""
    u = np.ascontiguousarray(x, dtype=np.float32).view(np.uint32)
    r = ((u + 0x7FFF + ((u >> 16) & 1)) >> 16).astype(np.uint16)
    return r.view(ml_dtypes.bfloat16)


def _build():
    nc = bacc.Bacc()
    enc = nc.declare_dram_parameter("enc", [BL, S, CTX], BF16, isOutput=False)
    enct = nc.declare_dram_parameter("enct", [BL, CTX, S], BF16, isOutput=False)
    ut = nc.declare_dram_parameter("ut", [CTX, A], BF16, isOutput=False)
    db = nc.declare_dram_parameter("db", [BL, A], BF16, isOutput=False)
    v = nc.declare_dram_parameter("vv", [A], BF16, isOutput=False)
    out = nc.declare_dram_parameter("out", [BL, CTX], F32, isOutput=True)

    ST = 512          # s-rows per super-tile (one ~1MB DMA per layout)
    NSUB = ST // P    # 4 proj subtiles per super-tile
    NSUP = S // ST    # 8 super-tiles per batch

    with tile.TileContext(nc) as tc:
        with (
            tc.tile_pool(name="const", bufs=1) as const,
            tc.tile_pool(name="work", bufs=4) as work,
            tc.tile_pool(name="psum", bufs=3, space="PSUM") as psum,
            tc.tile_pool(name="acc", bufs=1, space="PSUM") as accp,
        ):
            # ---- constants, loaded once ----
            ut_sb = const.tile([P, KC, A], BF16)
            nc.sync.dma_start(ut_sb[:], ut.rearrange("(k p) a -> p k a", p=P))
            db_sb = const.tile([1, BL * A], BF16)
            nc.sync.dma_start(db_sb[:], db.rearrange("b a -> (b a)")[None, :])
            v_sb = const.tile([1, A], BF16)
            nc.sync.dma_start(v_sb[:], v[None, :])
            ones_row = const.tile([1, P], BF16)
            nc.vector.memset(ones_row[:], 1.0)
            ones_col = const.tile([P, 1], BF16)
            nc.vector.memset(ones_col[:], 1.0)
            zbias0 = const.tile([P, 1], F32)
            nc.vector.memset(zbias0[:], 0.0)

            # v replicated to all 128 partitions: ones_row.T @ v_sb
            vrep_ps = psum.tile([P, A], F32, tag="proj")
            nc.tensor.matmul(vrep_ps[:], ones_row[:], v_sb[:], start=True, stop=True)
            # Walrus allows only ONE sync-wait per Activation instruction, so the
            # ScalarE stream is sequenced to observe the PE and DVE clocks up
            # front: (1) the v_rep copy waits on PE, (2) the zbias_act copy
            # waits on DVE.  The steady-state Tanh then needs only its PSUM
            # (PE) wait, and Exp's bias (zbias_act) keeps its deps on the ACT
            # semaphore where they merge into a single wait.
            v_rep = const.tile([P, A], F32)
            nc.scalar.activation(v_rep[:], vrep_ps[:],
                                 mybir.ActivationFunctionType.Copy)
            zbias = const.tile([P, 1], F32)
            nc.scalar.activation(zbias[:], zbias0[:],
                                 mybir.ActivationFunctionType.Copy)

            enct3 = enct.rearrange("b (k p) s -> b p k s", p=P)
            for b in range(BL):
                c0 = accp.tile([1, 512], F32, tag="c0")
                c1 = accp.tile([1, 512], F32, tag="c1")
                den = accp.tile([1, 1], F32, tag="den")
                for t in range(NSUP):
                    s0 = t * ST
                    nat = work.tile([P, NSUB, CTX], BF16, tag="nat")
                    nc.sync.dma_start(
                        nat[:],
                        enc[b, s0:s0 + ST, :].rearrange("(o p) c -> p o c", p=P))
                    tr = work.tile([P, KC, ST], BF16, tag="tr")
                    for u in range(NSUB):
                        for k in range(KC):
                            nc.sync.dma_start_transpose(
                                tr[:, k, u * P:(u + 1) * P],
                                nat[:, u, k * P:(k + 1) * P])

                    for u in range(NSUB):
                        j = t * NSUB + u
                        # projection + bias into PSUM [s=128, A]
                        proj = psum.tile([P, A], F32, tag="proj")
                        nc.tensor.matmul(
                            proj[:], ones_row[:], db_sb[:, b * A:(b + 1) * A],
                            start=True, stop=False,
                        )
                        for k in range(KC):
                            nc.tensor.matmul(
                                proj[:], tr[:, k, u * P:(u + 1) * P],
                                ut_sb[:, k, :],
                                start=False, stop=(k == KC - 1),
                            )

                        th = work.tile([P, A], F32, tag="th")
                        nc.scalar.activation(
                            th[:], proj[:], mybir.ActivationFunctionType.Tanh,
                            bias=zbias0[:],
                        )
                        ew = work.tile([P, A], F32, tag="ew")
                        nc.vector.tensor_mul(out=ew[:], in0=th[:], in1=v_rep[:])
                        dump = work.tile([P, A], BF16, tag="dump")
                        energy = work.tile([P, 1], F32, tag="energy")
                        nc.scalar.activation(
                            dump[:], ew[:], mybir.ActivationFunctionType.Copy,
                            accum_out=energy[:],
                        )
                        wexp = work.tile([P, 1], BF16, tag="wexp")
                        nc.scalar.activation(
                            wexp[:], energy[:], mybir.ActivationFunctionType.Exp,
                            bias=zbias[:],
                        )

                        first, last = (j == 0), (j == MT - 1)
                        nc.tensor.matmul(c0[:], wexp[:], nat[:, u, 0:512],
                                         start=first, stop=last)
                        nc.tensor.matmul(c1[:], wexp[:], nat[:, u, 512:1024],
                                         start=first, stop=last)
                        nc.tensor.matmul(den[:], wexp[:], ones_col[:],
                                         start=first, stop=last)

                rec = work.tile([1, 1], F32, tag="rec")
                nc.vector.reciprocal(rec[:], den[:])
                cout = work.tile([1, CTX], F32, tag="cout")
                nc.vector.tensor_scalar_mul(cout[:, 0:512], c0[:], rec[:])
                nc.vector.tensor_scalar_mul(cout[:, 512:1024], c1[:], rec[:])
                nc.sync.dma_start(out[b][None, :], cout[:])

    if not nc.is_finalized():
        nc.finalize()
    return nc


def kernel(previous_decoder_hidden_state, encoder_final_hidden_layers,
           W, b_W, U, b_U, v):
    prev = np.asarray(previous_decoder_hidden_state, dtype=np.float32)
    enc = np.asarray(encoder_final_hidden_layers, dtype=np.float32)
    W = np.asarray(W, dtype=np.float32)
    b_W = np.asarray(b_W, dtype=np.float32)
    U = np.asarray(U, dtype=np.float32)
    b_U = np.asarray(b_U, dtype=np.float32)
    v = np.asarray(v, dtype=np.float32)

    if "nc" not in _CACHE:
        _CACHE["nc"] = _build()
    nc = _CACHE["nc"]

    # host-side prep (tiny, except the enc cast which uses a fast bit path)
    db = (prev @ W.T + b_W + b_U).astype(ml_dtypes.bfloat16)   # [B, A]
    ut = np.ascontiguousarray(U.T).astype(ml_dtypes.bfloat16)  # [CTX, A]
    enc_bf = _fast_bf16(enc)                                   # [B, S, CTX]
    enct_bf = np.ascontiguousarray(enc_bf.transpose(0, 2, 1))  # [B, CTX, S]
    v_bf = v.astype(ml_dtypes.bfloat16)

    in_maps = []
    for i in range(NCORES):
        sl = slice(i * BL, (i + 1) * BL)
        in_maps.append({
            "enc": enc_bf[sl],
            "enct": enct_bf[sl],
            "ut": ut,
            "db": db[sl],
            "vv": v_bf,
        })

    res = run_bass_kernel_spmd(nc, in_maps, list(range(NCORES)),
                               **_CACHE.get("run_kwargs", {}))
    _CACHE["last_result"] = res
    outs = [np.asarray(r["out"]) for r in res.results]
    return np.concatenate(outs, axis=0).astype(np.float32)



# revision 11
# speedup vs baseline: 2.6859x; 2.6859x over previous
"""Bahdanau additive attention, data-parallel over batch on 8 TRN2 NeuronCores.

Math (per batch row b):
    dec_proj = W @ prev[b] + b_W                       # [A]   (host: tiny)
    enc_proj[s] = U @ enc[b,s] + b_U                   # [S, A]
    energy[s] = v . tanh(dec_proj + enc_proj[s])       # [S]
    w = exp(energy);  c[b] = (w @ enc[b]) / sum(w)     # [CTX]

All-bf16 compute (fp8 fails the 2e-2 accuracy budget: softmax is highly
concentrated, eff-n ~ 7).  Per core, 8 batches x 4096 s-rows:

  - enc natural bf16 loaded per 512-row super-tile as [s=128, 4, 1024]
    (ws matmul rhs).  Transposed side [c, s] needed as proj lhsT:
      * lower c-half comes PRE-TRANSPOSED from the host (32 MiB HBM)
      * upper c-half via multi-tile XBAR transpose [128,512]->[128,4,128]
        (one DMA per s-subtile; out layout c = k*128 + q, verified)
  - projection: 8 matmuls [c=128,s=128].T @ ut[c=128,A=256] -> psum[s,A],
    bias pre-written into PSUM by ScalarE (start=False accumulation).
  - tanh on ScalarE -> th bf16; energy = sum_a th*v via one DVE
    tensor_tensor_reduce; exp -> wbuf[:, j] bf16 (ScalarE).
  - weighted sum pipelined D=2 s-tiles behind the projection so the
    tanh->tt->exp chain never stalls PE:  c0/c1[1,512] += w.T @ nat.
  - den: one matmul ones.T @ wbuf -> [1,32] psum + DVE reduce per batch.
  - epilogue per batch: c = c_psum * (1/den), DMA out fp32.
"""

import sys

sys.path.insert(0, "/opt/trn_rl_repo")

import numpy as np
import ml_dtypes

import concourse.bass as bass
from concourse import bacc
import concourse.mybir as mybir
import concourse.tile as tile
from concourse.bass_utils import run_bass_kernel_spmd

B, S, A, DD, CTX = 64, 4096, 256, 1024, 1024
NCORES = 8
BL = B // NCORES   # 8 batches per core
P = 128
ST = 512           # s-rows per super-tile
NSUB = ST // P     # 4 s-subtiles per super-tile
NSUP = S // ST     # 8 super-tiles per batch
MT = S // P        # 32 s-tiles per batch
NT = BL * MT       # 256 s-tiles per core
NG = BL * NSUP     # 64 super-tiles per core
KC = CTX // P      # 8 contraction chunks
CH = CTX // 2      # host-transposed lower c-half
BF16 = mybir.dt.bfloat16
F32 = mybir.dt.float32

D = 2              # ws pipeline delay in s-tiles
PF_LOAD = 3        # super-tiles of load prefetch
PF_TR = 2          # super-tiles of transpose lead
USE_PREWRITE = False  # bias via ScalarE PSUM pre-write (else bias matmul)
USE_TT = False       # energy via DVE tensor_tensor_reduce (else mul + accum)

_CACHE = {}


def _fast_bf16(x: np.ndarray) -> np.ndarray:
    """float32 -> bfloat16 with round-to-nearest-even via integer ops."""
    u = np.ascontiguousarray(x, dtype=np.float32).view(np.uint32)
    r = ((u + 0x7FFF + ((u >> 16) & 1)) >> 16).astype(np.uint16)
    return r.view(ml_dtypes.bfloat16)


def _build():
    nc = bacc.Bacc()
    enc = nc.declare_dram_parameter("enc", [BL, S, CTX], BF16, isOutput=False)
    enct = nc.declare_dram_parameter("enct", [BL, NSUP, P, NSUB, KC // 2, P],
                                     BF16, isOutput=False)
    ut = nc.declare_dram_parameter("ut", [P, KC, A], BF16, isOutput=False)
    biasr = nc.declare_dram_parameter("biasr", [P, BL, A], F32, isOutput=False)
    dbrow = nc.declare_dram_parameter("dbrow", [1, BL * A], BF16, isOutput=False)
    vrep = nc.declare_dram_parameter("vrep", [P, A], BF16, isOutput=False)
    out = nc.declare_dram_parameter("out", [BL, CTX], F32, isOutput=True)

    with tile.TileContext(nc) as tc:
        with (
            tc.tile_pool(name="const", bufs=1) as const,
            tc.tile_pool(name="natp", bufs=5) as natp,
            tc.tile_pool(name="trhp", bufs=4) as trhp,
            tc.tile_pool(name="trxp", bufs=4) as trxp,
            tc.tile_pool(name="actp", bufs=3) as actp,
            tc.tile_pool(name="wbp", bufs=2) as wbp,
            tc.tile_pool(name="psproj", bufs=4, space="PSUM") as psproj,
            tc.tile_pool(name="psacc", bufs=1, space="PSUM") as psacc,
        ):
            # ---- constants ----
            ut_sb = const.tile([P, KC, A], BF16)
            nc.sync.dma_start(ut_sb[:], ut[:])
            bias_sb = const.tile([P, BL, A], F32)
            nc.sync.dma_start(bias_sb[:], biasr[:])
            v_sb = const.tile([P, A], BF16)
            nc.sync.dma_start(v_sb[:], vrep[:])
            ones_col = const.tile([P, 1], BF16)
            nc.vector.memset(ones_col[:], 1.0)
            db_sb = const.tile([1, BL * A], BF16)
            nc.sync.dma_start(db_sb[:], dbrow[:])
            ones_row = const.tile([1, P], BF16)
            nc.vector.memset(ones_row[:], 1.0)
            scr = const.tile([P, 1], BF16)  # tt_reduce dump (broadcast out)

            nat = {}    # g -> [128, NSUB, CTX] bf16
            trh = {}    # g -> [128, NSUB, KC//2, P] bf16 (c in [0, 512))
            trx = {}    # g -> [128, NSUB, KC//2, P] bf16 (c in [512, 1024))
            projps = {} # i -> [128, A] f32 psum
            wbuf = {}   # b -> [128, MT] bf16
            c0 = {}
            c1 = {}

            def issue_loads(g):
                if g >= NG:
                    return
                b, t = g // NSUP, g % NSUP
                nat[g] = natp.tile([P, NSUB, CTX], BF16, tag="nat", name=f"nat{g}")
                nc.sync.dma_start(
                    nat[g][:],
                    enc[b, t * ST:(t + 1) * ST, :].rearrange(
                        "(o p) c -> p o c", p=P))
                trh[g] = trhp.tile([P, NSUB, KC // 2, P], BF16, tag="trh", name=f"trh{g}")
                nc.sync.dma_start(trh[g][:], enct[b, t])

            def issue_transposes(g):
                if g >= NG:
                    return
                trx[g] = trxp.tile([P, NSUB, KC // 2, P], BF16, tag="trx", name=f"trx{g}")
                for o in range(NSUB):
                    nc.scalar.dma_start_transpose(
                        trx[g][:, o, :, :], nat[g][:, o, CH:CTX])

            def prewrite(i):
                if i >= NT:
                    return
                b = i // MT
                projps[i] = psproj.tile([P, A], F32, tag="proj", name=f"proj{i}")
                if USE_PREWRITE:
                    nc.scalar.activation(projps[i][:], bias_sb[:, b, :],
                                         mybir.ActivationFunctionType.Copy)
                else:
                    nc.tensor.matmul(projps[i][:], ones_row[:],
                                     db_sb[:, b * A:(b + 1) * A],
                                     start=True, stop=False)

            # ---- warmup ----
            for g in range(PF_LOAD):
                issue_loads(g)
            for g in range(PF_TR):
                issue_transposes(g)
            for i in range(D):
                prewrite(i)

            for i in range(NT + D):
                if i < NT:
                    b, jj = i // MT, i % MT
                    g, o = i // NSUB, i % NSUB
                    if o == 0:
                        issue_loads(g + PF_LOAD)
                        issue_transposes(g + PF_TR)
                    # ScalarE stream: prewrite(i+D), tanh(i), [exp(i-1) below]
                    prewrite(i + D)
                    # projection: 8 matmuls accumulate onto the bias
                    pp = projps[i]
                    for k in range(KC):
                        lhsT = (trh[g][:, o, k, :] if k < KC // 2
                                else trx[g][:, o, k - KC // 2, :])
                        nc.tensor.matmul(pp[:], lhsT, ut_sb[:, k, :],
                                         start=False, stop=(k == KC - 1),
                                         skip_group_check=True)
                    th = actp.tile([P, A], BF16, tag="th")
                    nc.scalar.activation(th[:], pp[:],
                                         mybir.ActivationFunctionType.Tanh)
                    en = actp.tile([P, 1], F32, tag="en")
                    if USE_TT:
                        nc.vector.tensor_tensor_reduce(
                            out=scr.broadcast_to(th[:].shape), in0=th[:],
                            in1=v_sb[:], scale=1.0, scalar=0.0,
                            op0=mybir.AluOpType.mult, op1=mybir.AluOpType.add,
                            accum_out=en[:])
                    else:
                        ew = actp.tile([P, A], BF16, tag="ew")
                        nc.vector.tensor_mul(out=ew[:], in0=th[:], in1=v_sb[:])
                        dump = actp.tile([P, A], BF16, tag="dump")
                        nc.scalar.activation(
                            dump[:], ew[:], mybir.ActivationFunctionType.Copy,
                            accum_out=en[:])
                    if jj == 0:
                        wbuf[b] = wbp.tile([P, MT], BF16, tag="wb", name=f"wb{b}")
                    nc.scalar.activation(wbuf[b][:, jj:jj + 1], en[:],
                                         mybir.ActivationFunctionType.Exp)

                iw = i - D
                if iw >= 0:
                    bw, jw = iw // MT, iw % MT
                    gw, ow = iw // NSUB, iw % NSUB
                    if jw == 0:
                        c0[bw] = psacc.tile([1, 512], F32, tag="c0", name=f"c0_{bw}")
                        c1[bw] = psacc.tile([1, 512], F32, tag="c1", name=f"c1_{bw}")
                    first, last = (jw == 0), (jw == MT - 1)
                    wcol = wbuf[bw][:, jw:jw + 1]
                    nc.tensor.matmul(c0[bw][:], wcol, nat[gw][:, ow, 0:512],
                                     start=first, stop=last)
                    nc.tensor.matmul(c1[bw][:], wcol, nat[gw][:, ow, 512:1024],
                                     start=first, stop=last)
                    if last:
                        den = psacc.tile([1, MT], F32, tag="den")
                        nc.tensor.matmul(den[:], ones_col[:], wbuf[bw][:],
                                         start=True, stop=True)
                        dsum = actp.tile([1, 1], F32, tag="dsum")
                        nc.vector.tensor_reduce(
                            dsum[:], den[:], axis=mybir.AxisListType.X,
                            op=mybir.AluOpType.add)
                        rec = actp.tile([1, 1], F32, tag="rec")
                        nc.vector.reciprocal(rec[:], dsum[:])
                        cout = actp.tile([1, CTX], F32, tag="cout")
                        nc.vector.tensor_scalar_mul(cout[:, 0:512], c0[bw][:],
                                                    rec[:])
                        nc.vector.tensor_scalar_mul(cout[:, 512:1024],
                                                    c1[bw][:], rec[:])
                        nc.sync.dma_start(out[bw][None, :], cout[:])

    if not nc.is_finalized():
        nc.finalize()
    return nc


def kernel(previous_decoder_hidden_state, encoder_final_hidden_layers,
           W, b_W, U, b_U, v):
    prev = np.asarray(previous_decoder_hidden_state, dtype=np.float32)
    enc = np.asarray(encoder_final_hidden_layers, dtype=np.float32)
    W = np.asarray(W, dtype=np.float32)
    b_W = np.asarray(b_W, dtype=np.float32)
    U = np.asarray(U, dtype=np.float32)
    b_U = np.asarray(b_U, dtype=np.float32)
    v = np.asarray(v, dtype=np.float32)

    if "nc" not in _CACHE:
        _CACHE["nc"] = _build()
    nc = _CACHE["nc"]

    # ---- host-side prep ----
    enc_bf = _fast_bf16(enc)                                  # [B, S, CTX]
    # host-transposed lower c-half: [b, t, q, o, k, j] = enc[b, t*512+o*128+j,
    #                                                        k*128+q]
    e = enc_bf[:, :, :CH].reshape(B, NSUP, NSUB, P, KC // 2, P)
    enct = np.ascontiguousarray(e.transpose(0, 1, 5, 2, 4, 3))
    UT = np.ascontiguousarray(U.T)                            # [CTX, A]
    ut_host = np.ascontiguousarray(
        UT.reshape(KC, P, A).transpose(1, 0, 2)).astype(ml_dtypes.bfloat16)
    db = prev @ W.T + b_W + b_U                               # [B, A] f32
    db_bf = db.astype(ml_dtypes.bfloat16)
    v_host = np.ascontiguousarray(
        np.broadcast_to(v[None, :], (P, A))).astype(ml_dtypes.bfloat16)

    in_maps = []
    for i in range(NCORES):
        sl = slice(i * BL, (i + 1) * BL)
        biasr = np.ascontiguousarray(
            np.broadcast_to(db[sl][None, :, :], (P, BL, A))).astype(np.float32)
        in_maps.append({
            "enc": enc_bf[sl],
            "enct": enct[sl],
            "ut": ut_host,
            "biasr": biasr,
            "dbrow": db_bf[sl].reshape(1, BL * A),
            "vrep": v_host,
        })

    res = run_bass_kernel_spmd(nc, in_maps, list(range(NCORES)),
                               **_CACHE.get("run_kwargs", {}))
    _CACHE["last_result"] = res
    outs = [np.asarray(r["out"]) for r in res.results]
    return np.concatenate(outs, axis=0).astype(np.float32)


# revision 17
# speedup vs baseline: 6.1091x; 2.2745x over previous
"""Bahdanau additive attention, data-parallel over batch on 8 TRN2 NeuronCores.

Math (per batch row b):
    dec_proj = W @ prev[b] + b_W                       # [A]   (host: tiny)
    enc_proj[s] = U @ enc[b,s] + b_U                   # [S, A]
    energy[s] = v . tanh(dec_proj + enc_proj[s])       # [S]
    w = exp(energy);  c[b] = (w @ enc[b]) / sum(w)     # [CTX]

All-bf16 compute (fp8 fails the 2e-2 accuracy budget: softmax is highly
concentrated, eff-n ~ 7).  Per core, 8 batches x 4096 s-rows:

  - enc natural bf16 loaded per 512-row super-tile as [s=128, 4, 1024]
    (ws matmul rhs).  Transposed side [c, s] needed as proj lhsT:
      * lower c-half comes PRE-TRANSPOSED from the host (32 MiB HBM)
      * upper c-half via multi-tile XBAR transpose [128,512]->[128,4,128]
        (one DMA per s-subtile; out layout c = k*128 + q, verified)
  - projection: 8 matmuls [c=128,s=128].T @ ut[c=128,A=256] -> psum[s,A],
    bias pre-written into PSUM by ScalarE (start=False accumulation).
  - tanh on ScalarE -> th bf16; energy = sum_a th*v via one DVE
    tensor_tensor_reduce; exp -> wbuf[:, j] bf16 (ScalarE).
  - weighted sum pipelined D=2 s-tiles behind the projection so the
    tanh->tt->exp chain never stalls PE:  c0/c1[1,512] += w.T @ nat.
  - den: one matmul ones.T @ wbuf -> [1,32] psum + DVE reduce per batch.
  - epilogue per batch: c = c_psum * (1/den), DMA out fp32.
"""

import sys

sys.path.insert(0, "/opt/trn_rl_repo")

import numpy as np
import ml_dtypes

import concourse.bass as bass
from concourse import bacc
import concourse.mybir as mybir
import concourse.tile as tile
from concourse.bass_utils import run_bass_kernel_spmd

B, S, A, DD, CTX = 64, 4096, 256, 1024, 1024
NCORES = 8
BL = B // NCORES   # 8 batches per core
P = 128
ST = 512           # s-rows per super-tile
NSUB = ST // P     # 4 s-subtiles per super-tile
NSUP = S // ST     # 8 super-tiles per batch
MT = S // P        # 32 s-tiles per batch
NT = BL * MT       # 256 s-tiles per core
NG = BL * NSUP     # 64 super-tiles per core
KC = CTX // P      # 8 contraction chunks
CH = CTX // 2      # host-transposed lower c-range (rest via XBAR)
KH = CH // P       # host-transposed k-chunks
BF16 = mybir.dt.bfloat16
F32 = mybir.dt.float32

D = 6              # ws pipeline delay in s-tiles
L = 2              # bias pre-emission lead (must be < psproj bufs - 1)
PF_LOAD = 3        # super-tiles of load prefetch
PF_TR = 2          # super-tiles of transpose lead
USE_PREWRITE = False  # bias via ScalarE PSUM pre-write (else bias matmul)
USE_TT = False       # energy via DVE tensor_tensor_reduce (else mul + accum)

_CACHE = {}


def _fast_bf16(x: np.ndarray) -> np.ndarray:
    """float32 -> bfloat16 with round-to-nearest-even via integer ops."""
    u = np.ascontiguousarray(x, dtype=np.float32).view(np.uint32)
    r = ((u + 0x7FFF + ((u >> 16) & 1)) >> 16).astype(np.uint16)
    return r.view(ml_dtypes.bfloat16)


def _build():
    nc = bacc.Bacc()
    enc = nc.declare_dram_parameter("enc", [BL, S, CTX], BF16, isOutput=False)
    enct = nc.declare_dram_parameter("enct", [BL, NSUP, P, NSUB, KH, P],
                                     BF16, isOutput=False)
    ut = nc.declare_dram_parameter("ut", [P, KC, A], BF16, isOutput=False)
    biasr = nc.declare_dram_parameter("biasr", [P, BL, A], F32, isOutput=False)
    dbrow = nc.declare_dram_parameter("dbrow", [1, BL * A], BF16, isOutput=False)
    vrep = nc.declare_dram_parameter("vrep", [P, A], BF16, isOutput=False)
    out = nc.declare_dram_parameter("out", [BL, CTX], F32, isOutput=True)

    with tile.TileContext(nc) as tc:
        with (
            tc.tile_pool(name="const", bufs=1) as const,
            tc.tile_pool(name="natp", bufs=7) as natp,
            tc.tile_pool(name="trhp", bufs=4) as trhp,
            tc.tile_pool(name="trxp", bufs=4) as trxp,
            tc.tile_pool(name="actp", bufs=3) as actp,
            tc.tile_pool(name="wbp", bufs=2) as wbp,
            tc.tile_pool(name="psproj", bufs=4, space="PSUM") as psproj,
            tc.tile_pool(name="psacc", bufs=1, space="PSUM") as psacc,
        ):
            # ---- constants ----
            ut_sb = const.tile([P, KC, A], BF16)
            nc.sync.dma_start(ut_sb[:], ut[:])
            bias_sb = const.tile([P, BL, A], F32)
            nc.sync.dma_start(bias_sb[:], biasr[:])
            v_sb = const.tile([P, A], BF16)
            nc.sync.dma_start(v_sb[:], vrep[:])
            ones_col = const.tile([P, 1], BF16)
            nc.vector.memset(ones_col[:], 1.0)
            db_sb = const.tile([1, BL * A], BF16)
            nc.sync.dma_start(db_sb[:], dbrow[:])
            ones_row = const.tile([1, P], BF16)
            nc.vector.memset(ones_row[:], 1.0)
            scr = const.tile([P, 1], BF16)  # tt_reduce dump (broadcast out)

            nat = {}    # g -> [128, NSUB, CTX] bf16
            trh = {}    # g -> [128, NSUB, KC//2, P] bf16 (c in [0, 512))
            trx = {}    # g -> [128, NSUB, KC//2, P] bf16 (c in [512, 1024))
            projps = {} # i -> [128, A] f32 psum
            en = {}     # g -> [128, NSUB] f32
            wbuf = {}   # b -> [128, MT] bf16
            c0 = {}
            c1 = {}

            def issue_loads(g):
                if g >= NG:
                    return
                b, t = g // NSUP, g % NSUP
                nat[g] = natp.tile([P, NSUB, CTX], BF16, tag="nat", name=f"nat{g}")
                nc.sync.dma_start(
                    nat[g][:],
                    enc[b, t * ST:(t + 1) * ST, :].rearrange(
                        "(o p) c -> p o c", p=P))
                trh[g] = trhp.tile([P, NSUB, KH, P], BF16, tag="trh", name=f"trh{g}")
                nc.sync.dma_start(trh[g][:], enct[b, t])

            def issue_transposes(g):
                if g >= NG or KH == KC:
                    return
                trx[g] = trxp.tile([P, NSUB, KC - KH, P], BF16, tag="trx", name=f"trx{g}")
                for o in range(NSUB):
                    nc.scalar.dma_start_transpose(
                        trx[g][:, o, :, :], nat[g][:, o, CH:CTX])

            def prewrite(i):
                if i >= NT:
                    return
                b = i // MT
                projps[i] = psproj.tile([P, A], F32, tag="proj", name=f"proj{i}")
                if USE_PREWRITE:
                    nc.scalar.activation(projps[i][:], bias_sb[:, b, :],
                                         mybir.ActivationFunctionType.Copy)
                else:
                    nc.tensor.matmul(projps[i][:], ones_row[:],
                                     db_sb[:, b * A:(b + 1) * A],
                                     start=True, stop=False)

            # ---- warmup ----
            for g in range(PF_LOAD):
                issue_loads(g)
            for g in range(PF_TR):
                issue_transposes(g)
            for i in range(L):
                prewrite(i)

            for i in range(NT + D):
                if i < NT:
                    b, jj = i // MT, i % MT
                    g, o = i // NSUB, i % NSUB
                    if o == 0:
                        issue_loads(g + PF_LOAD)
                        issue_transposes(g + PF_TR)
                    prewrite(i + L)
                    # projection: 8 matmuls accumulate onto the bias
                    pp = projps[i]
                    for k in range(KC):
                        lhsT = (trh[g][:, o, k, :] if k < KH
                                else trx[g][:, o, k - KH, :])
                        nc.tensor.matmul(pp[:], lhsT, ut_sb[:, k, :],
                                         start=False, stop=(k == KC - 1),
                                         skip_group_check=USE_PREWRITE)
                    th = actp.tile([P, A], BF16, tag="th")
                    nc.scalar.activation(th[:], pp[:],
                                         mybir.ActivationFunctionType.Tanh)
                    if o == 0:
                        en[g] = actp.tile([P, NSUB], F32, tag="en",
                                          name=f"en{g}")
                    ew = actp.tile([P, A], BF16, tag="ew")
                    nc.vector.tensor_mul(out=ew[:], in0=th[:], in1=v_sb[:])
                    nc.vector.tensor_reduce(
                        en[g][:, o:o + 1], ew[:], axis=mybir.AxisListType.X,
                        op=mybir.AluOpType.add)
                    if jj == 0:
                        wbuf[b] = wbp.tile([P, MT], BF16, tag="wb", name=f"wb{b}")
                    if o == NSUB - 1:
                        t = g % NSUP
                        nc.scalar.activation(
                            wbuf[b][:, t * NSUB:(t + 1) * NSUB], en[g][:],
                            mybir.ActivationFunctionType.Exp)

                iw = i - D
                if iw >= 0:
                    bw, jw = iw // MT, iw % MT
                    gw, ow = iw // NSUB, iw % NSUB
                    if jw == 0:
                        c0[bw] = psacc.tile([1, 512], F32, tag="c0", name=f"c0_{bw}")
                        c1[bw] = psacc.tile([1, 512], F32, tag="c1", name=f"c1_{bw}")
                    first, last = (jw == 0), (jw == MT - 1)
                    wcol = wbuf[bw][:, jw:jw + 1]
                    nc.tensor.matmul(c0[bw][:], wcol, nat[gw][:, ow, 0:512],
                                     start=first, stop=last)
                    nc.tensor.matmul(c1[bw][:], wcol, nat[gw][:, ow, 512:1024],
                                     start=first, stop=last)
                    if last:
                        den = psacc.tile([1, MT], F32, tag="den")
                        nc.tensor.matmul(den[:], ones_col[:], wbuf[bw][:],
                                         start=True, stop=True)
                        dsum = actp.tile([1, 1], F32, tag="dsum")
                        nc.vector.tensor_reduce(
                            dsum[:], den[:], axis=mybir.AxisListType.X,
                            op=mybir.AluOpType.add)
                        rec = actp.tile([1, 1], F32, tag="rec")
                        nc.vector.reciprocal(rec[:], dsum[:])
                        cout = actp.tile([1, CTX], F32, tag="cout")
                        nc.vector.tensor_scalar_mul(cout[:, 0:512], c0[bw][:],
                                                    rec[:])
                        nc.vector.tensor_scalar_mul(cout[:, 512:1024],
                                                    c1[bw][:], rec[:])
                        nc.sync.dma_start(out[bw][None, :], cout[:])

    if not nc.is_finalized():
        nc.finalize()
    return nc


def kernel(previous_decoder_hidden_state, encoder_final_hidden_layers,
           W, b_W, U, b_U, v):
    prev = np.asarray(previous_decoder_hidden_state, dtype=np.float32)
    enc = np.asarray(encoder_final_hidden_layers, dtype=np.float32)
    W = np.asarray(W, dtype=np.float32)
    b_W = np.asarray(b_W, dtype=np.float32)
    U = np.asarray(U, dtype=np.float32)
    b_U = np.asarray(b_U, dtype=np.float32)
    v = np.asarray(v, dtype=np.float32)

    if "nc" not in _CACHE:
        _CACHE["nc"] = _build()
    nc = _CACHE["nc"]

    # ---- host-side prep ----
    enc_bf = _fast_bf16(enc)                                  # [B, S, CTX]
    # host-transposed lower c-half: [b, t, q, o, k, j] = enc[b, t*512+o*128+j,
    #                                                        k*128+q]
    e = enc_bf[:, :, :CH].reshape(B, NSUP, NSUB, P, KH, P)
    enct = np.ascontiguousarray(e.transpose(0, 1, 5, 2, 4, 3))
    UT = np.ascontiguousarray(U.T)                            # [CTX, A]
    ut_host = np.ascontiguousarray(
        UT.reshape(KC, P, A).transpose(1, 0, 2)).astype(ml_dtypes.bfloat16)
    db = prev @ W.T + b_W + b_U                               # [B, A] f32
    db_bf = db.astype(ml_dtypes.bfloat16)
    v_host = np.ascontiguousarray(
        np.broadcast_to(v[None, :], (P, A))).astype(ml_dtypes.bfloat16)

    in_maps = []
    for i in range(NCORES):
        sl = slice(i * BL, (i + 1) * BL)
        biasr = np.ascontiguousarray(
            np.broadcast_to(db[sl][None, :, :], (P, BL, A))).astype(np.float32)
        in_maps.append({
            "enc": enc_bf[sl],
            "enct": enct[sl],
            "ut": ut_host,
            "biasr": biasr,
            "dbrow": db_bf[sl].reshape(1, BL * A),
            "vrep": v_host,
        })

    res = run_bass_kernel_spmd(nc, in_maps, list(range(NCORES)),
                               **_CACHE.get("run_kwargs", {}))
    _CACHE["last_result"] = res
    outs = [np.asarray(r["out"]) for r in res.results]
    return np.concatenate(outs, axis=0).astype(np.float32)


# revision 19
# speedup vs baseline: 6.5447x; 1.0713x over previous
"""Bahdanau additive attention, data-parallel over batch on 8 TRN2 NeuronCores.

Math (per batch row b):
    dec_proj = W @ prev[b] + b_W                       # [A]   (host: tiny)
    enc_proj[s] = U @ enc[b,s] + b_U                   # [S, A]
    energy[s] = v . tanh(dec_proj + enc_proj[s])       # [S]
    w = exp(energy);  c[b] = (w @ enc[b]) / sum(w)     # [CTX]

All-bf16 compute (fp8 fails the 2e-2 accuracy budget: softmax is highly
concentrated, eff-n ~ 7).  Per core, 8 batches x 4096 s-rows:

  - enc natural bf16 loaded per 512-row super-tile as [s=128, 4, 1024]
    (ws matmul rhs).  Transposed side [c, s] needed as proj lhsT:
      * lower c-half comes PRE-TRANSPOSED from the host (32 MiB HBM)
      * upper c-half via multi-tile XBAR transpose [128,512]->[128,4,128]
        (one DMA per s-subtile; out layout c = k*128 + q, verified)
  - projection: 8 matmuls [c=128,s=128].T @ ut[c=128,A=256] -> psum[s,A],
    bias pre-written into PSUM by ScalarE (start=False accumulation).
  - tanh on ScalarE -> th bf16; energy = sum_a th*v via one DVE
    tensor_tensor_reduce; exp -> wbuf[:, j] bf16 (ScalarE).
  - weighted sum pipelined D=2 s-tiles behind the projection so the
    tanh->tt->exp chain never stalls PE:  c0/c1[1,512] += w.T @ nat.
  - den: one matmul ones.T @ wbuf -> [1,32] psum + DVE reduce per batch.
  - epilogue per batch: c = c_psum * (1/den), DMA out fp32.
"""

import sys

sys.path.insert(0, "/opt/trn_rl_repo")

import numpy as np
import ml_dtypes

import concourse.bass as bass
from concourse import bacc
import concourse.mybir as mybir
import concourse.tile as tile
from concourse.bass_utils import run_bass_kernel_spmd

B, S, A, DD, CTX = 64, 4096, 256, 1024, 1024
NCORES = 8
BL = B // NCORES   # 8 batches per core
P = 128
ST = 512           # s-rows per super-tile
NSUB = ST // P     # 4 s-subtiles per super-tile
NSUP = S // ST     # 8 super-tiles per batch
MT = S // P        # 32 s-tiles per batch
NT = BL * MT       # 256 s-tiles per core
NG = BL * NSUP     # 64 super-tiles per core
KC = CTX // P      # 8 contraction chunks
CH = CTX // 2      # host-transposed lower c-range (rest via XBAR)
KH = CH // P       # host-transposed k-chunks
BF16 = mybir.dt.bfloat16
F32 = mybir.dt.float32

D = 6              # ws pipeline delay in s-tiles
L = 2              # bias pre-emission lead (must be < psproj bufs - 1)
PF_LOAD = 3        # super-tiles of load prefetch
PF_TR = 2          # super-tiles of transpose lead
USE_PREWRITE = True  # bias via ScalarE PSUM pre-write (else bias matmul)
USE_TT = False       # energy via DVE tensor_tensor_reduce (else mul + accum)

_CACHE = {}


def _fast_bf16(x: np.ndarray) -> np.ndarray:
    """float32 -> bfloat16 (RNE). jax CPU cast is multithreaded; fall back
    to a vectorized integer path if jax is unavailable."""
    try:
        import jax, jax.numpy as jnp
        with jax.default_device(jax.devices("cpu")[0]):
            return np.asarray(jnp.asarray(x).astype(jnp.bfloat16))
    except Exception:
        u = np.ascontiguousarray(x, dtype=np.float32).view(np.uint32)
        r = ((u + 0x7FFF + ((u >> 16) & 1)) >> 16).astype(np.uint16)
        return r.view(ml_dtypes.bfloat16)


def _build():
    nc = bacc.Bacc()
    enc = nc.declare_dram_parameter("enc", [BL, S, CTX], BF16, isOutput=False)
    enct = nc.declare_dram_parameter("enct", [BL, NSUP, P, NSUB, KH, P],
                                     BF16, isOutput=False)
    ut = nc.declare_dram_parameter("ut", [P, KC, A], BF16, isOutput=False)
    biasr = nc.declare_dram_parameter("biasr", [P, BL, A], F32, isOutput=False)
    dbrow = nc.declare_dram_parameter("dbrow", [1, BL * A], BF16, isOutput=False)
    vrep = nc.declare_dram_parameter("vrep", [P, A], BF16, isOutput=False)
    out = nc.declare_dram_parameter("out", [BL, CTX], F32, isOutput=True)

    with tile.TileContext(nc) as tc:
        with (
            tc.tile_pool(name="const", bufs=1) as const,
            tc.tile_pool(name="natp", bufs=7) as natp,
            tc.tile_pool(name="trhp", bufs=4) as trhp,
            tc.tile_pool(name="trxp", bufs=4) as trxp,
            tc.tile_pool(name="actp", bufs=3) as actp,
            tc.tile_pool(name="wbp", bufs=2) as wbp,
            tc.tile_pool(name="psproj", bufs=3, space="PSUM") as psproj,
            tc.tile_pool(name="psacc", bufs=2, space="PSUM") as psacc,
            tc.tile_pool(name="psden", bufs=1, space="PSUM") as psden,
        ):
            # ---- constants ----
            ut_sb = const.tile([P, KC, A], BF16)
            nc.sync.dma_start(ut_sb[:], ut[:])
            bias_sb = const.tile([P, BL, A], F32)
            nc.sync.dma_start(bias_sb[:], biasr[:])
            v_sb = const.tile([P, A], BF16)
            nc.sync.dma_start(v_sb[:], vrep[:])
            ones_col = const.tile([P, 1], BF16)
            nc.vector.memset(ones_col[:], 1.0)
            db_sb = const.tile([1, BL * A], BF16)
            nc.sync.dma_start(db_sb[:], dbrow[:])
            ones_row = const.tile([1, P], BF16)
            nc.vector.memset(ones_row[:], 1.0)
            scr = const.tile([P, 1], BF16)  # tt_reduce dump (broadcast out)

            nat = {}    # g -> [128, NSUB, CTX] bf16
            trh = {}    # g -> [128, NSUB, KC//2, P] bf16 (c in [0, 512))
            trx = {}    # g -> [128, NSUB, KC//2, P] bf16 (c in [512, 1024))
            projps = {} # i -> [128, A] f32 psum
            en = {}     # g -> [128, NSUB] f32
            wbuf = {}   # b -> [128, MT] bf16
            c0 = {}
            c1 = {}

            def issue_loads(g):
                if g >= NG:
                    return
                b, t = g // NSUP, g % NSUP
                nat[g] = natp.tile([P, NSUB, CTX], BF16, tag="nat", name=f"nat{g}")
                nc.sync.dma_start(
                    nat[g][:],
                    enc[b, t * ST:(t + 1) * ST, :].rearrange(
                        "(o p) c -> p o c", p=P))
                trh[g] = trhp.tile([P, NSUB, KH, P], BF16, tag="trh", name=f"trh{g}")
                nc.sync.dma_start(trh[g][:], enct[b, t])

            def issue_transposes(g):
                if g >= NG or KH == KC:
                    return
                trx[g] = trxp.tile([P, NSUB, KC - KH, P], BF16, tag="trx", name=f"trx{g}")
                for o in range(NSUB):
                    nc.scalar.dma_start_transpose(
                        trx[g][:, o, :, :], nat[g][:, o, CH:CTX])

            def prewrite(i):
                if i >= NT:
                    return
                b = i // MT
                projps[i] = psproj.tile([P, A], F32, tag="proj", name=f"proj{i}")
                if USE_PREWRITE:
                    nc.scalar.activation(projps[i][:], bias_sb[:, b, :],
                                         mybir.ActivationFunctionType.Copy)
                else:
                    nc.tensor.matmul(projps[i][:], ones_row[:],
                                     db_sb[:, b * A:(b + 1) * A],
                                     start=True, stop=False)

            # ---- warmup ----
            for g in range(PF_LOAD):
                issue_loads(g)
            for g in range(PF_TR):
                issue_transposes(g)
            for i in range(L):
                prewrite(i)

            for i in range(NT + D):
                if i < NT:
                    b, jj = i // MT, i % MT
                    g, o = i // NSUB, i % NSUB
                    if o == 0:
                        issue_loads(g + PF_LOAD)
                        issue_transposes(g + PF_TR)
                    prewrite(i + L)
                    # projection: 8 matmuls accumulate onto the bias
                    pp = projps[i]
                    for k in range(KC):
                        lhsT = (trh[g][:, o, k, :] if k < KH
                                else trx[g][:, o, k - KH, :])
                        nc.tensor.matmul(pp[:], lhsT, ut_sb[:, k, :],
                                         start=False, stop=(k == KC - 1),
                                         skip_group_check=USE_PREWRITE)
                    th = actp.tile([P, A], BF16, tag="th")
                    nc.scalar.activation(th[:], pp[:],
                                         mybir.ActivationFunctionType.Tanh)
                    if o == 0:
                        en[g] = actp.tile([P, NSUB], F32, tag="en",
                                          name=f"en{g}")
                    ew = actp.tile([P, A], BF16, tag="ew")
                    nc.vector.tensor_mul(out=ew[:], in0=th[:], in1=v_sb[:])
                    nc.vector.tensor_reduce(
                        en[g][:, o:o + 1], ew[:], axis=mybir.AxisListType.X,
                        op=mybir.AluOpType.add)
                    if jj == 0:
                        wbuf[b] = wbp.tile([P, MT], BF16, tag="wb", name=f"wb{b}")
                    if o == NSUB - 1:
                        t = g % NSUP
                        nc.scalar.activation(
                            wbuf[b][:, t * NSUB:(t + 1) * NSUB], en[g][:],
                            mybir.ActivationFunctionType.Exp)

                iw = i - D
                if iw >= 0:
                    bw, jw = iw // MT, iw % MT
                    gw, ow = iw // NSUB, iw % NSUB
                    if jw == 0:
                        c0[bw] = psacc.tile([1, 512], F32, tag="c0", name=f"c0_{bw}")
                        c1[bw] = psacc.tile([1, 512], F32, tag="c1", name=f"c1_{bw}")
                    first, last = (jw == 0), (jw == MT - 1)
                    wcol = wbuf[bw][:, jw:jw + 1]
                    nc.tensor.matmul(c0[bw][:], wcol, nat[gw][:, ow, 0:512],
                                     start=first, stop=last)
                    nc.tensor.matmul(c1[bw][:], wcol, nat[gw][:, ow, 512:1024],
                                     start=first, stop=last)
                    if last:
                        den = psden.tile([1, MT], F32, tag="den")
                        nc.tensor.matmul(den[:], ones_col[:], wbuf[bw][:],
                                         start=True, stop=True)
                        dsum = actp.tile([1, 1], F32, tag="dsum")
                        nc.vector.tensor_reduce(
                            dsum[:], den[:], axis=mybir.AxisListType.X,
                            op=mybir.AluOpType.add)
                        rec = actp.tile([1, 1], F32, tag="rec")
                        nc.vector.reciprocal(rec[:], dsum[:])
                        cout = actp.tile([1, CTX], F32, tag="cout")
                        nc.vector.tensor_scalar_mul(cout[:, 0:512], c0[bw][:],
                                                    rec[:])
                        nc.vector.tensor_scalar_mul(cout[:, 512:1024],
                                                    c1[bw][:], rec[:])
                        nc.sync.dma_start(out[bw][None, :], cout[:])

    if not nc.is_finalized():
        nc.finalize()
    return nc


def kernel(previous_decoder_hidden_state, encoder_final_hidden_layers,
           W, b_W, U, b_U, v):
    prev = np.asarray(previous_decoder_hidden_state, dtype=np.float32)
    enc = np.asarray(encoder_final_hidden_layers, dtype=np.float32)
    W = np.asarray(W, dtype=np.float32)
    b_W = np.asarray(b_W, dtype=np.float32)
    U = np.asarray(U, dtype=np.float32)
    b_U = np.asarray(b_U, dtype=np.float32)
    v = np.asarray(v, dtype=np.float32)

    if "nc" not in _CACHE:
        _CACHE["nc"] = _build()
    nc = _CACHE["nc"]

    # ---- host-side prep ----
    enc_bf = _fast_bf16(enc)                                  # [B, S, CTX]
    # host-transposed lower c-half: [b, t, q, o, k, j] = enc[b, t*512+o*128+j,
    #                                                        k*128+q]
    e = enc_bf[:, :, :CH].reshape(B, NSUP, NSUB, P, KH, P)
    enct = np.ascontiguousarray(e.transpose(0, 1, 5, 2, 4, 3))
    UT = np.ascontiguousarray(U.T)                            # [CTX, A]
    ut_host = np.ascontiguousarray(
        UT.reshape(KC, P, A).transpose(1, 0, 2)).astype(ml_dtypes.bfloat16)
    db = prev @ W.T + b_W + b_U                               # [B, A] f32
    db_bf = db.astype(ml_dtypes.bfloat16)
    v_host = np.ascontiguousarray(
        np.broadcast_to(v[None, :], (P, A))).astype(ml_dtypes.bfloat16)

    in_maps = []
    for i in range(NCORES):
        sl = slice(i * BL, (i + 1) * BL)
        biasr = np.ascontiguousarray(
            np.broadcast_to(db[sl][None, :, :], (P, BL, A))).astype(np.float32)
        in_maps.append({
            "enc": enc_bf[sl],
            "enct": enct[sl],
            "ut": ut_host,
            "biasr": biasr,
            "dbrow": db_bf[sl].reshape(1, BL * A),
            "vrep": v_host,
        })

    res = run_bass_kernel_spmd(nc, in_maps, list(range(NCORES)),
                               **_CACHE.get("run_kwargs", {}))
    _CACHE["last_result"] = res
    outs = [np.asarray(r["out"]) for r in res.results]
    return np.concatenate(outs, axis=0).astype(np.float32)
